# revision 1
# baseline (speedup 1.0000x reference)
"""Self-contained Bass/Trainium2 kernel for nn_Attention (B=4, N=2048, D=1024, H=16, dh=64).

Sharding: 8 cores = (batch b in 0..3) x (sequence half in 0..1).
Each core computes the full attention output for its 1024 rows of its batch:
full-sequence K/V are computed on-core (duplicated across the pair), so no
cross-core communication is needed. Host feeds x[b] with the core's own rows
last so one SPMD program serves all cores; softmax is order-invariant in j.

Numerics: matmuls in float32r (TF32-class single-pass mode), attention
weights and V in fp16, accumulation in fp32 PSUM. Softmax uses the
exp-sum-divide form without max subtraction (scores are O(1) here); the row
sums come free from a ones-column appended to V, and the 1/sum normalization
is fused into the PSUM-evacuation multiply.
"""

import sys
import numpy as np

sys.path.insert(0, "/opt/trn_rl_repo")

B, N, DIM = 4, 2048, 1024
HEADS, DH = 16, 64
SCALE = DH ** -0.5  # 0.125
NC = 8
HALF = N // 2  # rows per core

_compiled = None


def _build():
    import concourse.tile as tile
    from concourse import bacc, mybir
    from concourse.masks import make_identity

    f32 = mybir.dt.float32
    f32r = mybir.dt.float32r
    f16 = mybir.dt.float16
    EXP = mybir.ActivationFunctionType.Exp

    nc = bacc.Bacc("TRN2", target_bir_lowering=False, debug=False, num_devices=NC)

    X = nc.dram_tensor("x", (DIM, N), f32r, kind="ExternalInput").ap()
    WQKV = nc.dram_tensor("w_qkv", (DIM, 3 * DIM), f32r, kind="ExternalInput").ap()
    WOUT = nc.dram_tensor("w_out", (DIM, DIM), f32r, kind="ExternalInput").ap()
    BOUT = nc.dram_tensor("b_out", (DIM,), f32, kind="ExternalInput").ap()
    Y = nc.dram_tensor("y", (HALF, DIM), f32, kind="ExternalOutput").ap()

    CT = DIM // 128          # 8 contraction tiles over channels
    MT = DIM // 128          # 8 dim tiles for each of q,k,v
    ICH = 512                # i-chunk width for attention
    NCH = HALF // ICH        # 2 chunks
    JT = N // 128            # 16 j tiles
    SEC = 2                  # j-tiles per PSUM/exp section
    NSEC = JT // SEC         # 8 sections
    VW = DH + 1              # 65: v block width incl. ones column

    with tile.TileContext(nc) as tc:
        with tc.tile_pool(name="persist", bufs=1) as persist, \
             tc.tile_pool(name="wpool", bufs=3) as wpool:

            ident32 = persist.tile([128, 128], f32, tag="ident32")
            make_identity(nc, ident32[:])
            ident = persist.tile([128, 128], f32r, tag="ident")
            nc.vector.tensor_copy(ident[:], ident32[:])

            # bias broadcast to all partitions once
            bias_src = persist.tile([1, DIM], f32, tag="bias_src")
            nc.sync.dma_start(bias_src[:], BOUT.rearrange("(o d) -> o d", o=1))
            bias = persist.tile([128, DIM], f32, tag="bias")
            nc.gpsimd.partition_broadcast(bias[:], bias_src[0:1, :])

            kT = [persist.tile([128, N], f32r, tag="kT", bufs=MT, name=f"kT{m}")
                  for m in range(MT)]
            v_ext = [persist.tile([128, HEADS * VW], f16, tag="vext", bufs=JT,
                                  name=f"vext{t}") for t in range(JT)]
            qT = [persist.tile([128, HALF], f32r, tag="qT", bufs=MT, name=f"qT{m}")
                  for m in range(MT)]

            def w_col(base, m):
                """[128, 8, 128] view of w_qkv[:, base+m*128 : +128], channel tiles in free."""
                return WQKV[:, base + m * 128:base + (m + 1) * 128].rearrange(
                    "(t p) d -> p t d", p=128)

            def project(psA, wApool, xbT_h, w_base, m):
                """One [128, HALF] psum tile of W[:, m-block].T @ xT for this half."""
                wt = wApool.tile([128, DIM], f32r, tag="wA", name=f"wt{w_base}_{m}")
                nc.sync.dma_start(wt.rearrange("p (t d) -> p t d", d=128), w_col(w_base, m))
                ps = psA.tile([128, HALF], f32, tag="proj", bufs=2, name=f"proj{w_base}_{m}")
                for ct in range(CT):
                    for s in range(HALF // 512):
                        nc.tensor.matmul(ps[:, s * 512:(s + 1) * 512],
                                         wt[:, ct * 128:(ct + 1) * 128],
                                         xbT_h[ct][:, s * 512:(s + 1) * 512],
                                         start=(ct == 0), stop=(ct == CT - 1))
                return ps

            with tc.tile_pool(name="psA", bufs=1, space="PSUM") as psA, \
                 tc.tile_pool(name="wApool", bufs=2) as wApool, \
                 tc.tile_pool(name="stage", bufs=2) as stage:
                for h in (0, 1):
                    # ---- x^T half h arrives pre-transposed from the host shard ----
                    xbT_h = [stage.tile([128, HALF], f32r, tag="xbT", bufs=CT,
                                        name=f"xbT{h}_{ct}") for ct in range(CT)]
                    for ct in range(CT):
                        nc.sync.dma_start(
                            xbT_h[ct][:],
                            X[ct * 128:(ct + 1) * 128, h * HALF:(h + 1) * HALF])

                    # ---- kT projection ----
                    for m in range(MT):
                        ps = project(psA, wApool, xbT_h, DIM, m)
                        nc.vector.tensor_copy(kT[m][:, h * HALF:(h + 1) * HALF], ps[:])

                    # ---- v projection (dims-major), then transpose into v_ext (fp16) ----
                    for m in range(MT):
                        ps = project(psA, wApool, xbT_h, 2 * DIM, m)
                        vs = stage.tile([128, HALF], f32r, tag="vstage")
                        nc.vector.tensor_copy(vs[:], ps[:])
                        for nt in range(HALF // 128):
                            tp = psA.tile([128, 128], f32r, tag="tp", bufs=4)
                            nc.tensor.transpose(tp[:], vs[:, nt * 128:(nt + 1) * 128],
                                                ident[:])
                            dst = v_ext[h * (HALF // 128) + nt].rearrange(
                                "p (hh c) -> p hh c", c=VW)[:, 2 * m:2 * m + 2, 0:DH]
                            nc.vector.tensor_copy(dst, tp.rearrange("p (hh c) -> p hh c", c=DH))

                    # ---- qT projection last, so attention can start right after ----
                    if h == 1:
                        for m in range(MT):
                            ps = project(psA, wApool, xbT_h, 0, m)
                            nc.vector.tensor_copy(qT[m][:], ps[:])

                # ones columns of v_ext
                for t in range(JT):
                    ones_col = v_ext[t].rearrange("p (hh c) -> p hh c", c=VW)[:, :, DH:VW]
                    nc.gpsimd.memset(ones_col, 1.0)

            # ================= phase B: attention + output projection =================
            with tc.tile_pool(name="attn", bufs=1) as attn, \
                 tc.tile_pool(name="psB", bufs=1, space="PSUM") as psB:
                for ch in range(NCH):
                    isl = slice(ch * ICH, (ch + 1) * ICH)
                    ctx = [attn.tile([128, ICH], f32r, tag="ctx", bufs=12,
                                     name=f"ctx{ch}_{t}") for t in range(MT)]
                    for hp in range(HEADS // 2):
                        po = [psB.tile([65, ICH], f32, tag="po", bufs=2,
                                       name=f"po{ch}_{hp}_{p}") for p in range(2)]
                        ats = {}
                        # software pipeline: dots(sec) -> exp(sec); av(sec-1) after
                        # dots(sec) so the PE never head-of-line blocks on ACT.
                        for sec in range(NSEC):
                            pp = [psB.tile([128, SEC * 512], f32, tag="dots", bufs=3,
                                           name=f"dots{ch}_{hp}_{sec}_{p}")
                                  for p in range(2)]
                            for j2 in range(SEC):
                                jt = sec * SEC + j2
                                for p in range(2):
                                    nc.tensor.matmul(
                                        pp[p][:, j2 * 512:(j2 + 1) * 512],
                                        kT[hp][p * 64:(p + 1) * 64, jt * 128:(jt + 1) * 128],
                                        qT[hp][p * 64:(p + 1) * 64, isl],
                                        start=True, stop=True)
                            at = [attn.tile([128, SEC * 512], f16, tag="attnT", bufs=5,
                                            name=f"at{ch}_{hp}_{sec}_{p}")
                                  for p in range(2)]
                            for p in range(2):
                                nc.scalar.activation(at[p][:], pp[p][:], EXP,
                                                     bias=0.0, scale=SCALE)
                            ats[sec] = at

                            def av(s):
                                for j2 in range(SEC):
                                    jt = s * SEC + j2
                                    for p in range(2):
                                        hd = 2 * hp + p
                                        nc.tensor.matmul(
                                            po[p][:],
                                            v_ext[jt][:, hd * VW:(hd + 1) * VW],
                                            ats[s][p][:, j2 * 512:(j2 + 1) * 512],
                                            start=(jt == 0), stop=(jt == JT - 1))

                            if sec >= 1:
                                av(sec - 1)
                                del ats[sec - 1]
                        av(NSEC - 1)
                        # normalize by the ones-column sums, write into ctx (f32r)
                        for p in range(2):
                            rs = attn.tile([128, ICH], f32, tag="rs", bufs=4)
                            nc.vector.reciprocal(rs[0:1, :], po[p][64:65, :])
                            rb = attn.tile([128, ICH], f32, tag="rb", bufs=4)
                            nc.gpsimd.partition_broadcast(rb[:], rs[0:1, :])
                            nc.vector.tensor_mul(ctx[hp][p * 64:(p + 1) * 64, :],
                                                 po[p][0:64, :], rb[p * 64:(p + 1) * 64, :])
                    # out-projection for this chunk: yp holds both 512-wide e-halves
                    for ipair in range(ICH // 256):
                        yp = [psB.tile([128, DIM], f32, tag="dots", bufs=3,
                                       name=f"yp{ch}_{ipair}_{i}") for i in range(2)]
                        for ft in range(MT):
                            wo = wpool.tile([128, DIM], f32r, tag="wO",
                                            name=f"wo{ch}_{ipair}_{ft}")
                            nc.sync.dma_start(wo[:], WOUT[ft * 128:(ft + 1) * 128, :])
                            for i in range(2):
                                it = ipair * 2 + i
                                for ec in range(2):
                                    nc.tensor.matmul(
                                        yp[i][:, ec * 512:(ec + 1) * 512],
                                        ctx[ft][:, it * 128:(it + 1) * 128],
                                        wo[:, ec * 512:(ec + 1) * 512],
                                        start=(ft == 0), stop=(ft == MT - 1))
                        for i in range(2):
                            it = ipair * 2 + i
                            for ec in range(2):
                                ysb = attn.tile([128, 512], f32, tag="ysb", bufs=2)
                                nc.vector.tensor_add(ysb[:], yp[i][:, ec * 512:(ec + 1) * 512],
                                                     bias[:, ec * 512:(ec + 1) * 512])
                                nc.sync.dma_start(
                                    Y[ch * ICH + it * 128:ch * ICH + (it + 1) * 128,
                                      ec * 512:(ec + 1) * 512], ysb[:])

    nc.compile()
    return nc


def _get_compiled():
    global _compiled
    if _compiled is None:
        _compiled = _build()
    return _compiled


def kernel(x, w_qkv, w_out, b_out):
    from concourse.bass_utils import run_bass_kernel_spmd

    nc = _get_compiled()
    x = np.asarray(x, dtype=np.float32)
    w_qkv = np.ascontiguousarray(np.asarray(w_qkv, dtype=np.float32))
    w_out = np.ascontiguousarray(np.asarray(w_out, dtype=np.float32))
    b_out = np.asarray(b_out, dtype=np.float32)

    in_maps = []
    for c in range(NC):
        b, half = divmod(c, 2)
        other = x[b][(1 - half) * HALF:(2 - half) * HALF]
        mine = x[b][half * HALF:(half + 1) * HALF]
        xb = np.ascontiguousarray(np.concatenate([other, mine], axis=0).T)
        in_maps.append({"x": xb, "w_qkv": w_qkv, "w_out": w_out, "b_out": b_out})

    res = run_bass_kernel_spmd(nc, in_maps, core_ids=list(range(NC)))

    out = np.empty((B, N, DIM), dtype=np.float32)
    for c in range(NC):
        b, half = divmod(c, 2)
        out[b, half * HALF:(half + 1) * HALF] = res.results[c]["y"]
    return out



# revision 5
# speedup vs baseline: 1.5562x; 1.5562x over previous
"""Self-contained Bass/Trainium2 kernel for nn_Attention (B=4, N=2048, D=1024, H=16, dh=64).

Sharding: 8 cores = (batch b in 0..3) x (head-group g in 0..1, 8 heads each).
Each core computes q/k/v projections for its 8 heads over the full sequence
(no duplicated K/V work), runs attention, and produces a PARTIAL output
projection y_g = ctx_g @ w_out[512g:512g+512].  The host sums the two
head-group partials per batch and adds the bias.  This is the perfect
1/8 FLOP split: 17.2 GFLOP/core.

Numerics: everything fp16 (inputs converted host-side), fp32 PSUM
accumulation.  Softmax uses exp-sum-divide without max subtraction
(scores are O(1)); row sums come free from a ones-column appended to V
(option-A AV: out = V_extT @ at -> [dh+1, queries], row 64 = sums).
"""

import sys
import numpy as np

sys.path.insert(0, "/opt/trn_rl_repo")

B, N, DIM = 4, 2048, 1024
HEADS, DH = 16, 64
SCALE = DH ** -0.5  # 0.125
NC = 8
HG = HEADS // 2          # 8 heads per core
HD = HG * DH             # 512 inner dims per core
HALF = N // 2            # kept for test.py compat (not used for sharding)

_compiled = None


def _build():
    import concourse.tile as tile
    from concourse import bacc, mybir

    f32 = mybir.dt.float32
    f16 = mybir.dt.float16
    EXP = mybir.ActivationFunctionType.Exp

    nc = bacc.Bacc("TRN2", target_bir_lowering=False, debug=False, num_devices=NC)

    # host supplies: xT [DIM, N] fp16 (x[b] transposed), wqkv [DIM, 3*HD] fp16
    # (q|k|v column slices for this head group), wout [HD, DIM] fp16 (row
    # slice).  Output: y [N, DIM] fp16 partial (no bias).
    XT = nc.dram_tensor("xt", (DIM, N), f16, kind="ExternalInput").ap()
    WQKV = nc.dram_tensor("w_qkv", (DIM, 3 * HD), f16, kind="ExternalInput").ap()
    WOUT = nc.dram_tensor("w_out", (HD, DIM), f16, kind="ExternalInput").ap()
    Y = nc.dram_tensor("y", (N, DIM), f16, kind="ExternalOutput").ap()

    CT = DIM // 128   # 8 contraction tiles over input channels
    MT = HD // 128    # 4 dim tiles for each of q,k (dims-major)
    TT = N // 128     # 16 token tiles
    QC = 1024         # queries per exp granule
    NQC = N // QC     # 2
    VW = DH + 1       # 65: v block width incl. ones column

    with tile.TileContext(nc) as tc:
        with tc.tile_pool(name="persist", bufs=1) as persist:
            # ---- SBUF persistent tensors ----
            xsb = persist.tile([128, CT * N], f16, tag="xsb")          # 32KB/part
            wsb = persist.tile([128, CT * 3 * HD], f16, tag="wsb")     # 24KB/part
            wout_sb = persist.tile([128, MT * DIM], f16, tag="wout")   # 8KB/part
            kT = [persist.tile([128, N], f16, tag="kT", bufs=MT, name=f"kT{m}")
                  for m in range(MT)]
            qT = [persist.tile([128, N], f16, tag="qT", bufs=MT, name=f"qT{m}")
                  for m in range(MT)]
            v_ext = [persist.tile([128, HG * VW], f16, tag="vext", bufs=TT,
                                  name=f"vext{t}") for t in range(TT)]
            ctx_n = [persist.tile([128, N], f16, tag="ctxn", bufs=MT,
                                  name=f"ctxn{m}") for m in range(MT)]

            # ---- DMA loads ----
            nc.sync.dma_start(
                xsb.rearrange("p (t d) -> p t d", d=N),
                XT.rearrange("(t p) d -> p t d", p=128))
            nc.sync.dma_start(
                wsb.rearrange("p (t d) -> p t d", d=3 * HD),
                WQKV.rearrange("(t p) d -> p t d", p=128))
            nc.sync.dma_start(
                wout_sb.rearrange("p (t d) -> p t d", d=DIM),
                WOUT.rearrange("(t p) d -> p t d", p=128))

            xv = xsb.rearrange("p (t d) -> p t d", d=N)       # [128, CT, N]
            wv = wsb.rearrange("p (t d) -> p t d", d=3 * HD)  # [128, CT, 3*HD]

            with tc.tile_pool(name="proj", bufs=1, space="PSUM") as projp, \
                 tc.tile_pool(name="dots", bufs=1, space="PSUM") as dotsp, \
                 tc.tile_pool(name="ctxp", bufs=1, space="PSUM") as ctxpp, \
                 tc.tile_pool(name="stage", bufs=2) as stage:

                def proj_dims(dst, m, base, tc_, tag):
                    """dims-major: dst[:, tc_*512:+512] = W[:, base+m*128:+128].T @ xT."""
                    ps = projp.tile([128, 512], f32, tag="proj", bufs=2,
                                    name=f"pp_{tag}{m}_{tc_}")
                    for ct in range(CT):
                        nc.tensor.matmul(ps[:],
                                         wv[:, ct, base + m * 128:base + (m + 1) * 128],
                                         xv[:, ct, tc_ * 512:(tc_ + 1) * 512],
                                         start=(ct == 0), stop=(ct == CT - 1))
                    nc.vector.tensor_copy(dst[:, tc_ * 512:(tc_ + 1) * 512], ps[:])

                def proj_v(tt):
                    """tokens-major V: v_ext[tt][:, h*65:h*65+64] = (xT tile).T @ w_v."""
                    ps = projp.tile([128, 512], f32, tag="proj", bufs=2,
                                    name=f"pp_v{tt}")
                    for ct in range(CT):
                        nc.tensor.matmul(ps[:],
                                         xv[:, ct, tt * 128:(tt + 1) * 128],
                                         wv[:, ct, 2 * HD:3 * HD],
                                         start=(ct == 0), stop=(ct == CT - 1))
                    dst = v_ext[tt].rearrange("p (h c) -> p h c", c=VW)[:, :, 0:DH]
                    nc.vector.tensor_copy(dst, ps.rearrange("p (h c) -> p h c", c=DH))

                # V first (AV needs every v_ext tile), then K0/Q0 so head 0
                # can start; remaining K/Q tiles interleave as PE fill work.
                for tt in range(TT):
                    proj_v(tt)
                for t_ in range(4):
                    proj_dims(kT[0], 0, HD, t_, "k")
                for t_ in range(4):
                    proj_dims(qT[0], 0, 0, t_, "q")

                # ones columns of v_ext
                for t in range(TT):
                    ones_col = v_ext[t].rearrange("p (h c) -> p h c", c=VW)[:, :, DH:VW]
                    nc.gpsimd.memset(ones_col, 1.0)

                # fill-task schedule: K/Q projections for dim-tile m+1 are
                # emitted between attention granules of head pair m, so they
                # complete (in PE program order) before the pair that reads
                # them starts.
                def pair_fills(pair):
                    m = pair + 1
                    if m >= MT:
                        return []
                    return ([(kT[m], m, HD, t_, "k") for t_ in range(4)]
                            + [(qT[m], m, 0, t_, "q") for t_ in range(4)])

                # ---- attention ----
                for h in range(HG):
                    m, p = h // 2, h % 2
                    rows = slice(p * 64, (p + 1) * 64)
                    if p == 0:
                        fills = pair_fills(h // 2)
                        fill_i = 0
                        granule = 0
                        # 2 heads x NQC x TT granules to absorb len(fills)
                        fill_every = max(1, (2 * NQC * TT) // (len(fills) + 1))
                    for qc in range(NQC):
                        qsl = slice(qc * QC, (qc + 1) * QC)
                        # ctx psum [65, 1024]: accumulated over all key tiles
                        cps = ctxpp.tile([65, QC], f32, tag="ctx", bufs=1,
                                         name=f"ctx{h}_{qc}")
                        for jt in range(TT):
                            # dots [128 keys, 1024 queries]
                            dp = dotsp.tile([128, QC], f32, tag="dots", bufs=2,
                                            name=f"d{h}_{qc}_{jt}")
                            for qs in range(QC // 512):
                                nc.tensor.matmul(
                                    dp[:, qs * 512:(qs + 1) * 512],
                                    kT[m][rows, jt * 128:(jt + 1) * 128],
                                    qT[m][rows, qc * QC + qs * 512:
                                          qc * QC + (qs + 1) * 512],
                                    start=True, stop=True)
                            at = stage.tile([128, QC], f16, tag="at", bufs=3,
                                            name=f"at{h}_{qc}_{jt}")
                            nc.scalar.activation(at[:], dp[:], EXP,
                                                 bias=0.0, scale=SCALE)
                            for qs in range(QC // 512):
                                nc.tensor.matmul(
                                    cps[:, qs * 512:(qs + 1) * 512],
                                    v_ext[jt][:, h * VW:(h + 1) * VW],
                                    at[:, qs * 512:(qs + 1) * 512],
                                    start=(jt == 0), stop=(jt == TT - 1))
                            granule += 1
                            if granule % fill_every == 0 and fill_i < len(fills):
                                fd, fm, fb, ft, tg = fills[fill_i]
                                proj_dims(fd, fm, fb, ft, tg)
                                fill_i += 1
                        # evacuate ctx psum quickly (frees banks), then
                        # normalize off-psum: ctx_n = ctx[0:64] / ctx[64]
                        craw = stage.tile([65, QC], f32, tag="craw", bufs=2,
                                          name=f"craw{h}_{qc}")
                        nc.vector.tensor_copy(craw[:], cps[:])
                        rs = stage.tile([1, QC], f32, tag="rs", bufs=2)
                        nc.vector.reciprocal(rs[:], craw[64:65, :])
                        rb = stage.tile([64, QC], f32, tag="rb", bufs=2)
                        nc.gpsimd.partition_broadcast(rb[:], rs[0:1, :])
                        nc.vector.tensor_mul(ctx_n[m][rows, qsl],
                                             craw[0:64, :], rb[:])
                    if p == 1:
                        while fill_i < len(fills):
                            fd, fm, fb, ft, tg = fills[fill_i]
                            proj_dims(fd, fm, fb, ft, tg)
                            fill_i += 1

            # ---- output projection (partial: this head-group only) ----
            wo = wout_sb.rearrange("p (t d) -> p t d", d=DIM)  # [128, MT, DIM]
            with tc.tile_pool(name="psB", bufs=1, space="PSUM") as psB, \
                 tc.tile_pool(name="ysb", bufs=3) as ysbp:
                for tt in range(TT):
                    yp = psB.tile([128, DIM], f32, tag="yp", bufs=2,
                                  name=f"yp{tt}")
                    for m in range(MT):
                        for ec in range(2):
                            nc.tensor.matmul(
                                yp[:, ec * 512:(ec + 1) * 512],
                                ctx_n[m][:, tt * 128:(tt + 1) * 128],
                                wo[:, m, ec * 512:(ec + 1) * 512],
                                start=(m == 0), stop=(m == MT - 1))
                    ys = ysbp.tile([128, DIM], f16, tag="ys", name=f"ys{tt}")
                    nc.vector.tensor_copy(ys[:], yp[:])
                    nc.sync.dma_start(Y[tt * 128:(tt + 1) * 128, :], ys[:])

    nc.compile()
    return nc


def _get_compiled():
    global _compiled
    if _compiled is None:
        _compiled = _build()
    return _compiled


def make_in_maps(x, w_qkv, w_out):
    """Per-core input maps (shared by kernel() and test profiling)."""
    x = np.asarray(x, dtype=np.float32)
    w_qkv = np.asarray(w_qkv, dtype=np.float32)
    w_out = np.asarray(w_out, dtype=np.float32)
    in_maps = []
    xts = [np.ascontiguousarray(x[b].T.astype(np.float16)) for b in range(B)]
    wq_slices = []
    wo_slices = []
    for g in range(2):
        cols = np.concatenate([
            w_qkv[:, 512 * g: 512 * (g + 1)],
            w_qkv[:, 1024 + 512 * g: 1024 + 512 * (g + 1)],
            w_qkv[:, 2048 + 512 * g: 2048 + 512 * (g + 1)],
        ], axis=1).astype(np.float16)
        wq_slices.append(np.ascontiguousarray(cols))
        wo_slices.append(np.ascontiguousarray(
            w_out[512 * g: 512 * (g + 1), :].astype(np.float16)))
    for c in range(NC):
        b, g = divmod(c, 2)
        in_maps.append({"xt": xts[b], "w_qkv": wq_slices[g],
                        "w_out": wo_slices[g]})
    return in_maps


def kernel(x, w_qkv, w_out, b_out):
    from concourse.bass_utils import run_bass_kernel_spmd

    nc = _get_compiled()
    in_maps = make_in_maps(x, w_qkv, w_out)
    res = run_bass_kernel_spmd(nc, in_maps, core_ids=list(range(NC)))

    b_out = np.asarray(b_out, dtype=np.float32)
    out = np.empty((B, N, DIM), dtype=np.float32)
    for b in range(B):
        out[b] = (res.results[2 * b]["y"].astype(np.float32)
                  + res.results[2 * b + 1]["y"].astype(np.float32) + b_out)
    return out
